# revision 14
# baseline (speedup 1.0000x reference)
"""Multi-head attention (B=4, P=2048, D=1024, H=16) on 8 TRN2 NeuronCores.

Sharding: tensor-parallel over heads (2 heads per core). Each core computes
qkv for its heads, full attention for its heads, and a partial output
projection (rows of w_proj for its heads). Partials are summed on host.

v6: single ACT-bound sweep per batch. Score matmuls for the two heads are
issued as row-group pairs (lhsT base partitions 0/64 -> tile_position rows
0/64) so they run concurrently in the PE array (measured 1.73x). One Exp
activation covers both heads' [128,512] score tiles ([128,1024], 2 banks).
Attention-value product keeps the [v | ones] M=65 stationary per head (the
ones column accumulates softmax denominators). qkv (K=128, N=1024), V
transposes, per-window normalization, and the output projection (both heads
merged into one K=128, N=1024 matmul) are emitted as filler work units
between sweep steps so the PE never idles and HAM stays at K=8/8.
"""

import numpy as np
import ml_dtypes

import concourse.bass as bass
import concourse.tile as tile
from concourse import bacc, mybir
from concourse import bass_utils
from concourse.masks import make_identity

B, P, D = 4, 2048, 1024
H = 16
NCORES = 8
HPC = H // NCORES          # heads per core = 2
d = D // H                 # 64
R = B * P                  # 8192
SCALE = float(d) ** -0.5

F32 = mybir.dt.float32
F32R = mybir.dt.float32r
BF16 = mybir.dt.bfloat16
AF = mybir.ActivationFunctionType

_CACHE = {}


def _build():
    nc = bacc.Bacc("TRN2", target_bir_lowering=False, debug=False,
                   enable_asserts=False)
    xT = nc.dram_tensor("xT", (D, R), BF16, kind="ExternalInput").ap()
    wqkv = nc.dram_tensor("wqkv", (128, 3072), BF16, kind="ExternalInput").ap()
    wproj = nc.dram_tensor("wproj", (128, D), BF16, kind="ExternalInput").ap()
    out = nc.dram_tensor("out", (R, D), F32, kind="ExternalOutput").ap()

    xT3 = xT.rearrange("(kb p) n -> p kb n", p=128)      # [128, 8, 8192]
    out3 = out.rearrange("(r p) n -> p r n", p=128)      # [128, 64, 1024]

    with tile.TileContext(nc) as tc:
        from contextlib import ExitStack
        from collections import deque
        with ExitStack() as ctx:
            p_const = ctx.enter_context(tc.tile_pool(name="const", bufs=1))
            p_w = ctx.enter_context(tc.tile_pool(name="w", bufs=1))
            p_x = ctx.enter_context(tc.tile_pool(name="x", bufs=2))
            p_qk = ctx.enter_context(tc.tile_pool(name="qk", bufs=2))
            p_v = ctx.enter_context(tc.tile_pool(name="v", bufs=2))
            p_vt = ctx.enter_context(tc.tile_pool(name="vt", bufs=2))
            p_e = ctx.enter_context(tc.tile_pool(name="e", bufs=4))
            p_ot = ctx.enter_context(tc.tile_pool(name="ot", bufs=2))
            p_otu = ctx.enter_context(tc.tile_pool(name="otu", bufs=2))
            p_bc = ctx.enter_context(tc.tile_pool(name="bc", bufs=4))
            p_out = ctx.enter_context(tc.tile_pool(name="o", bufs=2))
            # PSUM: 4 (scores, one kb-pair tile) + 2 (attnV accum)
            # + 2 (filler) = 8 banks
            ps_s = ctx.enter_context(
                tc.tile_pool(name="pss", bufs=1, space="PSUM"))
            ps_o = ctx.enter_context(
                tc.tile_pool(name="pso", bufs=1, space="PSUM"))
            ps_big = ctx.enter_context(
                tc.tile_pool(name="psb", bufs=1, space="PSUM"))

            ident = p_const.tile([128, 128], BF16)
            make_identity(nc, ident[:])
            # ones row at partition 64 (matches oTu denominator row)
            ones1f = p_const.tile([65, 64], F32)
            nc.vector.memset(ones1f[:], 1.0)
            ones1 = p_const.tile([65, 64], F32R)
            nc.vector.tensor_copy(ones1[:], ones1f[:])

            wq_sb = p_w.tile([128, 3072], BF16)
            nc.sync.dma_start(wq_sb[:], wqkv[:])
            wp_sb = p_w.tile([128, D], BF16)
            nc.sync.dma_start(wp_sb[:], wproj[:])

            # ---------- per-batch state ----------
            def alloc_batch(b):
                st = {}
                st["qt"] = p_qk.tile([128, P], BF16, tag="qt", name="qt")
                st["kt"] = p_qk.tile([128, P], BF16, tag="kt", name="kt")
                st["vON"] = [p_v.tile([128, 16 * 65], BF16, tag=f"v{h}",
                                      name=f"vON{h}_{b}") for h in range(2)]
                for h in range(2):
                    ov = st["vON"][h].rearrange("p (blk w) -> p blk w", w=65)
                    nc.vector.memset(ov[:, :, 64:65], 1.0)
                st["oTu"] = [p_otu.tile([65, P], F32R, tag=f"otu{h}",
                                        name=f"oTu{h}_{b}") for h in range(2)]
                st["oTn"] = p_ot.tile([128, P], BF16, tag="otn",
                                      name=f"oTn_{b}")
                st["xt"] = [None, None]
                return st

            def dma_x(st, b, cc):
                xt = p_x.tile([128, 8 * 1024], BF16, tag=f"x{cc}", name=f"x{cc}")
                for kb in range(8):
                    nc.sync.dma_start(
                        xt[:, kb * 1024:(kb + 1) * 1024],
                        xT3[:, kb, (b * 2 + cc) * 1024:(b * 2 + cc + 1) * 1024])
                st["xt"][cc] = xt

            # ---------- filler work units ----------
            queue = deque()

            def emit(k):
                while k > 0 and queue:
                    queue.popleft()()
                    k -= 1

            def u_qkv(st, cc, m, half):
                def go():
                    if half == 0:
                        st[f"qkvps{cc}{m}"] = ps_big.tile(
                            [128, 1024], F32, tag="big", name="qkvps")
                    ps = st[f"qkvps{cc}{m}"]
                    xt = st["xt"][cc]
                    for kb in range(half * 4, half * 4 + 4):
                        col = kb * 384 + m * 128
                        for nh in range(2):
                            nc.tensor.matmul(
                                ps[:, nh * 512:(nh + 1) * 512],
                                wq_sb[:, col:col + 128],
                                xt[:, kb * 1024 + nh * 512:
                                   kb * 1024 + (nh + 1) * 512],
                                start=(kb == 0), stop=(kb == 7))
                    if half == 0:
                        return
                    sl = slice(cc * 1024, (cc + 1) * 1024)
                    if m == 0:
                        nc.vector.tensor_copy(st["qt"][:, sl], ps[:])
                    elif m == 1:
                        nc.vector.tensor_copy(st["kt"][:, sl], ps[:])
                    else:
                        vt = p_vt.tile([128, 1024], BF16, tag="vt", name="vt")
                        nc.vector.tensor_copy(vt[:], ps[:])
                        st[f"vt{cc}"] = vt
                return go

            def u_transp(st, cc, rs):
                # one rs-chunk x 2 heads; each transpose gets its own psum
                # tile at offset 0 (multi-offset bf16-psum writes fault on HW)
                def go():
                    vt = st[f"vt{cc}"]
                    for h in range(2):
                        pt = ps_big.tile([128, 64], BF16, tag="big",
                                         name="pt")
                        nc.tensor.transpose(
                            pt[:],
                            vt[h * 64:(h + 1) * 64,
                               rs * 128:(rs + 1) * 128],
                            ident[h * 64:(h + 1) * 64,
                                  h * 64:(h + 1) * 64])
                        jb = cc * 8 + rs
                        nc.vector.tensor_copy(
                            st["vON"][h][:, jb * 65:jb * 65 + 64], pt[:])
                return go

            def u_norm(st, qw, h, drain=False):
                def go():
                    sl = slice(qw * 512, (qw + 1) * 512)
                    pool, tag = (ps_s, "s") if drain else (ps_big, "big")
                    ps = pool.tile([128, 1024], F32, tag=tag, name="nrmps")
                    nc.tensor.matmul(ps[0:64, 0:512], ones1[64:65, :],
                                     st["oTu"][h][64:65, sl],
                                     start=True, stop=True)
                    bcs = p_bc.tile([64, 512], F32, tag="bc", name="bcs")
                    nc.vector.reciprocal_approx_fast(bcs[:], ps[0:64, 0:512])
                    nc.vector.tensor_mul(
                        st["oTn"][h * 64:(h + 1) * 64, sl],
                        st["oTu"][h][0:64, sl], bcs[:])
                return go

            def u_proj(st, b, rr, drain=False):
                def go():
                    pool, tag = (ps_s, "s") if drain else (ps_big, "big")
                    ps = pool.tile([128, 1024], F32, tag=tag, name="prjps")
                    for nh in range(2):
                        nc.tensor.matmul(
                            ps[:, nh * 512:(nh + 1) * 512],
                            st["oTn"][:, rr * 128:(rr + 1) * 128],
                            wp_sb[:, nh * 512:(nh + 1) * 512],
                            start=True, stop=True)
                    outsb = p_out.tile([128, 1024], F32, tag="os", name="outsb")
                    nc.vector.tensor_copy(outsb[:], ps[:])
                    r0 = b * 16 + rr
                    nc.sync.dma_start(
                        out3[:, r0:r0 + 1, :],
                        outsb.rearrange("p (r n) -> p r n", n=1024))
                return go

            def push_stage_a(st, b):
                for cc in range(2):
                    for m in range(3):
                        queue.append(u_qkv(st, cc, m, 0))
                        queue.append(u_qkv(st, cc, m, 1))
                    for rs in range(8):
                        queue.append(u_transp(st, cc, rs))

            # ---------- the sweep ----------
            def sweep(st, b):
                qt, kt, vON = st["qt"], st["kt"], st["vON"]
                for qw in range(4):
                    q0 = qw * 512
                    psos = [ps_o.tile([65, 512], F32, tag=f"o{h}",
                                      name=f"pso{h}_{b}_{qw}")
                            for h in range(2)]
                    ets = [None] * 16

                    def attnv(kb):
                        for h in range(2):
                            nc.tensor.matmul(
                                psos[h], vON[h][:, kb * 65:(kb + 1) * 65],
                                ets[kb][:, h * 512:(h + 1) * 512],
                                start=(kb == 0), stop=(kb == 15))

                    # supersteps of 2 key blocks: all four score matmuls
                    # write one 4-bank psum tile, then a single wide Exp
                    # covers the pair; lagged attnV kb's fill the PE while
                    # the activation drains the psum tile
                    for ss in range(8):
                        pss = ps_s.tile([128, 2048], F32, tag="s",
                                        name="pss")
                        et = p_e.tile([128, 2048], BF16, tag="e",
                                      name="et")
                        for j, kb in enumerate((2 * ss, 2 * ss + 1)):
                            for h in range(2):
                                c0 = j * 1024 + h * 512
                                nc.tensor.matmul(
                                    pss[:, c0:c0 + 512],
                                    kt[h * 64:(h + 1) * 64,
                                       kb * 128:(kb + 1) * 128],
                                    qt[h * 64:(h + 1) * 64, q0:q0 + 512],
                                    start=True, stop=True)
                            ets[kb] = et[:, j * 1024:(j + 1) * 1024]
                        nc.scalar.activation(et[:], pss[:], AF.Exp,
                                             scale=SCALE)
                        if ss > 0:
                            attnv(2 * ss - 2)
                            attnv(2 * ss - 1)
                        emit(2 if len(queue) > 12 else 1)
                    attnv(14)
                    attnv(15)
                    for h in range(2):
                        nc.vector.tensor_copy(
                            st["oTu"][h][:, q0:q0 + 512], psos[h])
                    emit(1)
                    # normalization + projection for this window become
                    # filler units (popped during subsequent steps)
                    drain = (b == B - 1 and qw == 3)
                    queue.append(u_norm(st, qw, 0, drain))
                    queue.append(u_norm(st, qw, 1, drain))
                    for rr in range(qw * 4, qw * 4 + 4):
                        queue.append(u_proj(st, b, rr, drain))

            # ---------- main schedule ----------
            st = alloc_batch(0)
            dma_x(st, 0, 0)
            dma_x(st, 0, 1)
            # batch 0 stage A runs unoverlapped (prologue)
            push_stage_a(st, 0)
            emit(len(queue))
            states = {0: st}
            for b in range(B):
                if b + 1 < B:
                    nst = alloc_batch(b + 1)
                    dma_x(nst, b + 1, 0)
                    dma_x(nst, b + 1, 1)
                    push_stage_a(nst, b + 1)
                    states[b + 1] = nst
                sweep(states[b], b)
                states.pop(b - 1, None)
            emit(len(queue))

    nc.compile()
    return nc


def _in_maps(x, w_qkv, w_proj):
    x2 = np.ascontiguousarray(x.reshape(R, D).T)          # (D, R)
    xbf = x2.astype(ml_dtypes.bfloat16)
    Wq = w_qkv.reshape(D, 3, H, d)
    Wp = w_proj.reshape(H, d, D)
    maps = []
    for c in range(NCORES):
        hs = slice(c * HPC, (c + 1) * HPC)
        w_shard = np.ascontiguousarray(Wq[:, :, hs, :]).reshape(D, 3 * HPC * d)
        wq_pre = np.ascontiguousarray(
            w_shard.reshape(8, 128, 3, 128).transpose(1, 0, 2, 3)
        ).reshape(128, 3072)
        wp_shard = np.ascontiguousarray(Wp[hs]).reshape(HPC * d, D)
        maps.append({
            "xT": xbf,
            "wqkv": np.ascontiguousarray(wq_pre).astype(ml_dtypes.bfloat16),
            "wproj": wp_shard.astype(ml_dtypes.bfloat16),
        })
    return maps


def get_nc():
    if "nc" not in _CACHE:
        _CACHE["nc"] = _build()
    return _CACHE["nc"]


def kernel(x, w_qkv, w_proj, b_proj):
    x = np.asarray(x)
    w_qkv = np.asarray(w_qkv)
    w_proj = np.asarray(w_proj)
    b_proj = np.asarray(b_proj)
    nc = get_nc()
    maps = _in_maps(x, w_qkv, w_proj)
    res = bass_utils.run_bass_kernel_spmd(nc, maps, core_ids=list(range(NCORES)))
    acc = np.zeros((R, D), dtype=np.float64)
    for r in res.results:
        acc += r["out"].astype(np.float64)
    acc += b_proj.astype(np.float64)
    return acc.reshape(B, P, D).astype(np.float32)

